# revision 6
# baseline (speedup 1.0000x reference)
"""Trainium2 Bass kernel for CustomPatchEmbedding.

out[b,n,e] = sum_k patch(b,n)[k] * W[e,k] + bias[e], patches are 16x16x3
windows of x at (start_h, start_w)[b,n].

8 NeuronCores, data-parallel over batch (8 images/core).

Host prep:
 - x cast to fp16 and re-laid as all 368 sliding-window positions in
   HWC order: xb[b, wb, h, 48] = x[b, :, h, wb:wb+16] (HWC) -> a patch
   starting at (sh, wb) is the 768 CONTIGUOUS fp16 values at element
   offset ((b*368 + wb)*384 + sh)*48.
 - one int32 element-offset per token
 - weights prepacked to the SBUF-resident layout [128, KC*E] fp16 where
   wk[p, c*E+e] = W[(ph,pw,c)-major k = c*128+p, e] (single linear DMA)

Device per 128-token block:
 - indirect DMA gather: 128 rows x 1536 B fp16 -> dense patches
   [128 tok, 768] (one row per partition).
 - 6 PE transposes (fp16 in, fp16 psum out) -> patchesT [cpp, tok]
 - 6x2 accumulating fp16 matmuls vs resident weights -> psum [tok, 768]
 - DVE adds bias (f32) + casts fp16 -> SBUF, DMA out fp16.
A few dummy matmuls at t=0 warm the PE HAM clock gate while the DMAs
load, so the real matmuls run at 2.4 GHz from block 0. Host upcasts
the fp16 output to f32.
"""
import numpy as np

import concourse.bass as bass
import concourse.bacc as bacc
import concourse.mybir as mybir
import concourse.tile as tile
from concourse.bass_utils import run_bass_kernel_spmd

B, C, H, W = 64, 3, 384, 384
N, E, P = 576, 768, 16
NCORES = 8
BPC = B // NCORES          # 8 images per core
TOK = BPC * N              # 4608 tokens per core
CPP = C * P * P            # 768
KC = CPP // 128            # 6
BLK = 128
NBLK = TOK // BLK          # 36
NB = W - P + 1             # 368 sliding-window positions
BW = P * C                 # 48 fp16 per window row

f32 = mybir.dt.float32
fp16 = mybir.dt.float16
i32 = mybir.dt.int32

_cached = {}


def build_nc():
    nc = bacc.Bacc(trn_type="TRN2")
    xb = nc.dram_tensor("xb", [BPC * NB * H, BW], fp16, kind="ExternalInput")
    idx = nc.dram_tensor("idx", [128, NBLK], i32, kind="ExternalInput")
    wk = nc.dram_tensor("wk", [128, KC * E], fp16, kind="ExternalInput")
    biasr = nc.dram_tensor("biasr", [128, E], f32, kind="ExternalInput")
    ident_d = nc.dram_tensor("ident", [128, 128], fp16, kind="ExternalInput")
    out = nc.dram_tensor("out", [TOK, E], fp16, kind="ExternalOutput")
    warm_d = nc.dram_tensor("warm", [128, 8], f32, kind="ExternalOutput")

    with tile.TileContext(nc) as tc:
        with (
            tc.tile_pool(name="const", bufs=1) as cpool,
            tc.tile_pool(name="io", bufs=6) as iopool,
            tc.tile_pool(name="psumt", bufs=2, space="PSUM") as psumt_pool,
            tc.tile_pool(name="psumo", bufs=3, space="PSUM") as psumo_pool,
        ):
            idx_s = cpool.tile([128, NBLK], i32)
            nc.sync.dma_start(idx_s, idx[:, :])
            ident = cpool.tile([128, 128], fp16)
            nc.sync.dma_start(ident, ident_d[:, :])
            wk_s = cpool.tile([128, KC * E], fp16)
            nc.sync.dma_start(wk_s, wk[:, :])
            bias_s = cpool.tile([128, E], f32)
            nc.sync.dma_start(bias_s, biasr[:, :])

            # HAM warmup: ~4.3us of dummy matmuls with no DMA deps so the
            # PE clock gate opens while the real data loads.
            warm_src = cpool.tile([128, 512], fp16)
            nc.vector.memset(warm_src[:], 0)
            warm_psum = psumo_pool.tile([128, E], f32, tag="psum_out")
            for _ in range(13):
                nc.tensor.matmul(
                    warm_psum[:, 0:512], warm_src[:, 0:128], warm_src[:],
                    start=True, stop=True,
                )
            warm_s = cpool.tile([128, 8], f32)
            nc.vector.tensor_copy(warm_s[:], warm_psum[:, 0:8])
            nc.sync.dma_start(warm_d[:, :], warm_s[:])

            # Software-pipelined epilogue: block n's bias-add + store are
            # issued AFTER block n+1's psum->SBUF copy, so on the DVE's
            # in-order queue the copy (which the next matmuls wait on)
            # isn't stuck behind the slower add.
            pending = None   # (psum_out, blk) awaiting bias-add + store

            def flush_pending():
                nonlocal pending
                if pending is None:
                    return
                p_out, p_blk = pending
                out_s = iopool.tile([128, E], fp16, tag="out_s")
                nc.vector.tensor_add(out_s[:], p_out[:], bias_s[:])
                nc.sync.dma_start(
                    out[p_blk * BLK:(p_blk + 1) * BLK, :], out_s[:]
                )
                pending = None

            for blk in range(NBLK):
                patches = iopool.tile([128, CPP], fp16)
                nc.gpsimd.indirect_dma_start(
                    out=patches[:, :],
                    out_offset=None,
                    in_=xb[:, :],
                    in_offset=bass.IndirectOffsetOnAxis(
                        ap=idx_s[:, blk:blk + 1], axis=1
                    ),
                )
                psum_t = psumt_pool.tile([128, CPP], fp16)
                for kc in range(KC):
                    nc.tensor.transpose(
                        out=psum_t[:, kc * 128:(kc + 1) * 128],
                        in_=patches[:, kc * 128:(kc + 1) * 128],
                        identity=ident[:],
                    )
                patchesT = iopool.tile([128, CPP], fp16)
                nc.vector.tensor_copy(patchesT[:], psum_t[:])
                flush_pending()

                psum_out = psumo_pool.tile([128, E], f32, tag="psum_out")
                for kc in range(KC):
                    lhsT = patchesT[:, kc * 128:(kc + 1) * 128]
                    nc.tensor.matmul(
                        psum_out[:, 0:512],
                        lhsT,
                        wk_s[:, kc * E: kc * E + 512],
                        start=(kc == 0), stop=(kc == KC - 1),
                    )
                    nc.tensor.matmul(
                        psum_out[:, 512:768],
                        lhsT,
                        wk_s[:, kc * E + 512:(kc + 1) * E],
                        start=(kc == 0), stop=(kc == KC - 1),
                    )
                pending = (psum_out, blk)
            flush_pending()
    nc.finalize()
    return nc


def _prep_core_inputs(x_bands, start_h, start_w, wk_np, bias_rep, ident_np, core):
    b0 = core * BPC
    xc = x_bands[b0:b0 + BPC].reshape(-1, BW)
    sh = start_h[b0:b0 + BPC].reshape(TOK).astype(np.int64)
    sw = start_w[b0:b0 + BPC].reshape(TOK).astype(np.int64)
    img = np.repeat(np.arange(BPC, dtype=np.int64), N)
    idx = ((img * NB + sw) * H + sh) * BW
    return {
        "xb": xc,
        "idx": idx.astype(np.int32).reshape(NBLK, 128).T.copy(),
        "wk": wk_np,
        "biasr": bias_rep,
        "ident": ident_np,
    }


def kernel(x, start_h, start_w, proj_w, proj_b, _run_kwargs=None, _return_res=False):
    x = np.asarray(x, dtype=np.float32)
    start_h = np.asarray(start_h, dtype=np.int32)
    start_w = np.asarray(start_w, dtype=np.int32)
    proj_w = np.asarray(proj_w, dtype=np.float32)
    proj_b = np.asarray(proj_b, dtype=np.float32)

    # [B,H,W,C] fp16, then all 368 sliding windows: [B, NB, H, 16*C]
    x_hwc = np.ascontiguousarray(x.transpose(0, 2, 3, 1)).astype(np.float16)
    wins = np.lib.stride_tricks.sliding_window_view(x_hwc, P, axis=2)
    # wins[b, h, wb, c, dw] -> want [b, wb, h, dw, c]
    x_bands = np.ascontiguousarray(
        wins[:, :, :NB].transpose(0, 2, 1, 4, 3)
    ).reshape(B, NB, H, BW)

    w_km = proj_w.transpose(2, 3, 1, 0).reshape(CPP, E)        # (ph,pw,c),E
    wk_np = np.ascontiguousarray(
        w_km.reshape(KC, 128, E).transpose(1, 0, 2).reshape(128, KC * E)
    ).astype(np.float16)
    bias_rep = np.ascontiguousarray(np.broadcast_to(proj_b[None, :], (128, E)))
    ident_np = np.eye(128, dtype=np.float16)

    if "nc" not in _cached:
        _cached["nc"] = build_nc()
    nc = _cached["nc"]

    in_maps = [
        _prep_core_inputs(x_bands, start_h, start_w, wk_np, bias_rep, ident_np, c)
        for c in range(NCORES)
    ]
    res = run_bass_kernel_spmd(
        nc, in_maps, core_ids=list(range(NCORES)), **(_run_kwargs or {})
    )
    out = np.concatenate(
        [r["out"].reshape(BPC, N, E) for r in res.results], axis=0
    ).astype(np.float32)
    if _return_res:
        return out, res
    return out
